# revision 2
# baseline (speedup 1.0000x reference)
"""Trainium2 Bass kernel for nn_ChannelEmbeddingLayers (embedding_lookup).

Strategy: data-parallel over the batch (4096 rows -> 8 cores x 512), embedding
tables replicated per core. Per core, each 128-row batch tile does 200
indirect-DMA gathers (one per feature column, 128 rows each), DVE tree-pools
the sequence features, PE transposes the 2432-wide concat and matmuls against
the replicated dense layer (bias folded in as a constant-1 feature), ACT
applies ReLU straight out of PSUM.

kernel(**inputs) takes the full unsharded inputs and returns the full
[4096, 128] float32 output.
"""
import numpy as np
import concourse.bass as bass
import concourse.mybir as mybir
from concourse import bass_utils
from concourse.masks import make_identity

# Problem shapes (hardcoded per task spec).
B = 4096
N_CORES = 8
BS = B // N_CORES          # 512 rows per core
P = 128                    # batch tile rows
NT = BS // P               # 4 tiles per core
DF, SF = 70, 130           # dense / sparse feature counts
DD, SD = 64, 32            # dense / sparse embedding dims
DSC, SSC = 20, 30          # scalar (non-pooled) feature counts
DSQ, SSQ = DF - DSC, SF - SSC   # 50 / 100 pooled sequence features
V = 1_000_000
# concat layout: [dense scalars 1280][dense pooled 64][sparse scalars 960]
# [sparse pooled 32][ones 1][pad 95] -> K = 2432 = 19*128
OFF_DP = DSC * DD            # 1280
OFF_SS = OFF_DP + DD         # 1344
OFF_SP = OFF_SS + SSC * SD   # 2304
OFF_ONE = OFF_SP + SD        # 2336
K = 2432
KT = K // 128                # 19 contraction tiles
NOUT = 128

_cached = None


def _build():
    nc = bass.Bass()
    dkeys = nc.dram_tensor("dkeys", [BS, DF], mybir.dt.int32, kind="ExternalInput")
    skeys = nc.dram_tensor("skeys", [BS, SF], mybir.dt.int32, kind="ExternalInput")
    dtab = nc.dram_tensor("dtab", [V, DD], mybir.dt.float32, kind="ExternalInput")
    stab = nc.dram_tensor("stab", [V, SD], mybir.dt.float32, kind="ExternalInput")
    wmat = nc.dram_tensor("wmat", [K, NOUT], mybir.dt.float32, kind="ExternalInput")
    outp = nc.dram_tensor("outp", [BS, NOUT], mybir.dt.float32, kind="ExternalOutput")

    f32 = mybir.dt.float32
    i32 = mybir.dt.int32
    from contextlib import ExitStack
    with ExitStack() as ctx:
        e = ctx.enter_context
        kd = [e(nc.sbuf_tensor(f"kd{s}", [P, DF], i32)) for s in range(2)]
        ks = [e(nc.sbuf_tensor(f"ks{s}", [P, SF], i32)) for s in range(2)]
        X = [e(nc.sbuf_tensor(f"X{s}", [P, K], f32)) for s in range(2)]
        XT = [e(nc.sbuf_tensor(f"XT{s}", [P, K], f32)) for s in range(2)]
        dsq = [e(nc.sbuf_tensor(f"dsq{s}", [P, DSQ * DD], f32)) for s in range(2)]
        ssq = [e(nc.sbuf_tensor(f"ssq{s}", [P, SSQ * SD], f32)) for s in range(2)]
        W = e(nc.sbuf_tensor("W", [P, K], f32))
        ident = e(nc.sbuf_tensor("ident", [P, P], f32))
        osb = [e(nc.sbuf_tensor(f"osb{s}", [P, NOUT], f32)) for s in range(2)]
        tp = [e(nc.psum_tensor(f"tp{s}", [P, P], f32)) for s in range(2)]
        acc = [e(nc.psum_tensor(f"acc{s}", [P, NOUT], f32)) for s in range(2)]

        # one-shot DMA boundary sems (exact, race-free across the 16 SDMA engines)
        bd = [e(nc.semaphore(f"bd{t}")) for t in range(NT)]
        bs_ = [e(nc.semaphore(f"bs{t}")) for t in range(NT)]
        bx = [e(nc.semaphore(f"bx{t}")) for t in range(NT)]
        kt = [e(nc.semaphore(f"kt{t}")) for t in range(NT)]
        g_dump = e(nc.semaphore("g_dump"))
        w_sem = e(nc.semaphore("w_sem"))
        init_sem = e(nc.semaphore("init_sem"))
        pool_sem = e(nc.semaphore("pool_sem"))
        pe_t_sem = e(nc.semaphore("pe_t_sem"))
        act_c_sem = e(nc.semaphore("act_c_sem"))
        pe_m_sem = e(nc.semaphore("pe_m_sem"))
        act_r_sem = e(nc.semaphore("act_r_sem"))
        store_sem = e(nc.semaphore("store_sem"))

        block = e(nc.Block())

        @block.sync
        def _(sync):
            # replicated dense layer: 19 chunks of [128, 128]
            for k in range(KT):
                inst = sync.dma_start(W[:, k * 128:(k + 1) * 128],
                                      wmat[k * 128:(k + 1) * 128, :])
                inst.then_inc(w_sem, 16)
            for t in range(NT):
                if t >= 2:
                    sync.wait_ge(bx[t - 2], 16)  # keys slot free
                sync.dma_start(kd[t % 2][:], dkeys[t * P:(t + 1) * P, :]).then_inc(g_dump, 16)
                sync.dma_start(ks[t % 2][:], skeys[t * P:(t + 1) * P, :]).then_inc(kt[t], 16)
            for t in range(NT):
                sync.wait_ge(act_r_sem, t + 1)
                sync.dma_start(outp[t * P:(t + 1) * P, :], osb[t % 2][:]).then_inc(store_sem, 16)
            sync.wait_ge(store_sem, 16 * NT)

        @block.gpsimd
        def _(gpsimd):
            for s in range(2):
                gpsimd.memset(X[s][:, OFF_ONE:OFF_ONE + 1], 1.0)
                gpsimd.memset(X[s][:, OFF_ONE + 1:K], 0.0)
            make_identity(nc, ident[:])
            gpsimd.sem_inc(init_sem, 1)
            for t in range(NT):
                sl = t % 2
                gpsimd.wait_ge(kt[t], 16)
                if t >= 2:
                    gpsimd.wait_ge(pool_sem, 2 * (t - 1))   # dsq/ssq slot free
                    gpsimd.wait_ge(pe_t_sem, KT * (t - 1))  # X slot free

                def gat(dst, table, keys_sb, col, sem, val):
                    gpsimd.indirect_dma_start(
                        out=dst, out_offset=None, in_=table[:],
                        in_offset=bass.IndirectOffsetOnAxis(
                            ap=keys_sb[:, col:col + 1], axis=0),
                    ).then_inc(sem, val)

                for j in range(DSQ):   # dense sequence -> dsq
                    last = j == DSQ - 1
                    gat(dsq[sl][:, j * DD:(j + 1) * DD], dtab, kd[sl], DSC + j,
                        bd[t] if last else g_dump, 16)
                for j in range(SSQ):   # sparse sequence -> ssq
                    last = j == SSQ - 1
                    gat(ssq[sl][:, j * SD:(j + 1) * SD], stab, ks[sl], SSC + j,
                        bs_[t] if last else g_dump, 16)
                for j in range(DSC):   # dense scalars -> X
                    gat(X[sl][:, j * DD:(j + 1) * DD], dtab, kd[sl], j, g_dump, 16)
                for j in range(SSC):   # sparse scalars -> X
                    last = j == SSC - 1
                    gat(X[sl][:, OFF_SS + j * SD:OFF_SS + (j + 1) * SD], stab, ks[sl], j,
                        bx[t] if last else g_dump, 16)

        @block.vector
        def _(vector):
            for t in range(NT):
                sl = t % 2
                d, s = dsq[sl], ssq[sl]
                if t >= 2:
                    vector.wait_ge(pe_t_sem, KT * (t - 1))  # X slot free for pooled writes
                vector.wait_ge(bd[t], 16)
                # 50 chunks of 64 -> 1
                nc.vector.tensor_add(d[:, 0:1600], d[:, 0:1600], d[:, 1600:3200])
                nc.vector.tensor_add(d[:, 0:768], d[:, 0:768], d[:, 768:1536])
                nc.vector.tensor_add(d[:, 0:384], d[:, 0:384], d[:, 384:768])
                nc.vector.tensor_add(d[:, 0:192], d[:, 0:192], d[:, 192:384])
                nc.vector.tensor_add(d[:, 0:64], d[:, 0:64], d[:, 64:128])
                nc.vector.tensor_add(d[:, 0:64], d[:, 0:64], d[:, 128:192])
                nc.vector.tensor_add(d[:, 0:64], d[:, 0:64], d[:, 1536:1600])
                nc.vector.tensor_scalar_mul(
                    X[sl][:, OFF_DP:OFF_DP + DD], d[:, 0:64], 1.0 / DSQ
                ).then_inc(pool_sem, 1)
                vector.wait_ge(bs_[t], 16)
                # 100 chunks of 32 -> 1
                nc.vector.tensor_add(s[:, 0:1600], s[:, 0:1600], s[:, 1600:3200])
                nc.vector.tensor_add(s[:, 0:800], s[:, 0:800], s[:, 800:1600])
                nc.vector.tensor_add(s[:, 0:384], s[:, 0:384], s[:, 384:768])
                nc.vector.tensor_add(s[:, 0:192], s[:, 0:192], s[:, 192:384])
                nc.vector.tensor_add(s[:, 0:96], s[:, 0:96], s[:, 96:192])
                nc.vector.tensor_add(s[:, 0:32], s[:, 0:32], s[:, 32:64])
                nc.vector.tensor_add(s[:, 0:32], s[:, 0:32], s[:, 64:96])
                nc.vector.tensor_add(s[:, 0:32], s[:, 0:32], s[:, 768:800])
                nc.vector.tensor_scalar_mul(
                    X[sl][:, OFF_SP:OFF_SP + SD], s[:, 0:32], 1.0 / SSQ
                ).then_inc(pool_sem, 1)

        @block.tensor
        def _(tensor):
            tensor.wait_ge(init_sem, 1)
            tensor.wait_ge(w_sem, 16 * KT)
            for t in range(NT):
                sl = t % 2
                tensor.wait_ge(bx[t], 16)
                tensor.wait_ge(pool_sem, 2 * t + 2)
                for k in range(KT):
                    thresh = KT * t + k - 1
                    if thresh > 0:
                        tensor.wait_ge(act_c_sem, thresh)  # tp bank k%2 free
                    nc.tensor.transpose(
                        out=tp[k % 2][:], in_=X[sl][:, k * 128:(k + 1) * 128],
                        identity=ident[:],
                    ).then_inc(pe_t_sem, 1)
                tensor.wait_ge(act_c_sem, KT * (t + 1))  # all XT chunks ready
                if t >= 2:
                    tensor.wait_ge(act_r_sem, t - 1)     # acc slot free
                for k in range(KT):
                    inst = nc.tensor.matmul(
                        out=acc[t % 2][:],
                        lhsT=XT[sl][:, k * 128:(k + 1) * 128],
                        rhs=W[:, k * 128:(k + 1) * 128],
                        start=(k == 0), stop=(k == KT - 1),
                    )
                    if k == KT - 1:
                        inst.then_inc(pe_m_sem, 1)

        @block.scalar
        def _(scalar):
            for t in range(NT):
                sl = t % 2
                if t >= 2:
                    scalar.wait_ge(pe_m_sem, t - 1)      # XT slot free
                for k in range(KT):
                    scalar.wait_ge(pe_t_sem, KT * t + k + 1)
                    nc.scalar.copy(
                        out=XT[sl][:, k * 128:(k + 1) * 128], in_=tp[k % 2][:]
                    ).then_inc(act_c_sem, 1)
                scalar.wait_ge(pe_m_sem, t + 1)
                if t >= 2:
                    scalar.wait_ge(store_sem, 16 * (t - 1))  # osb slot free
                nc.scalar.activation(
                    out=osb[t % 2][:], in_=acc[t % 2][:],
                    func=mybir.ActivationFunctionType.Relu,
                ).then_inc(act_r_sem, 1)

    nc.finalize()
    return nc


def kernel(dense_keys, sparse_keys, dense_table, sparse_table, dnn_w, dnn_b):
    global _cached
    if _cached is None:
        _cached = _build()
    nc = _cached

    dense_keys = np.asarray(dense_keys)
    sparse_keys = np.asarray(sparse_keys)
    dense_table = np.ascontiguousarray(np.asarray(dense_table, dtype=np.float32))
    sparse_table = np.ascontiguousarray(np.asarray(sparse_table, dtype=np.float32))
    dnn_w = np.asarray(dnn_w, dtype=np.float32)
    dnn_b = np.asarray(dnn_b, dtype=np.float32)

    # fold bias in as a constant-1 feature; zero-pad K to 19*128
    wfull = np.zeros((K, NOUT), np.float32)
    wfull[:OFF_ONE] = dnn_w
    wfull[OFF_ONE] = dnn_b

    dk = np.ascontiguousarray(dense_keys.astype(np.int32))
    sk = np.ascontiguousarray(sparse_keys.astype(np.int32))

    in_maps = []
    for c in range(N_CORES):
        r = slice(c * BS, (c + 1) * BS)
        in_maps.append({
            "dkeys": dk[r], "skeys": sk[r],
            "dtab": dense_table, "stab": sparse_table,
            "wmat": wfull,
        })

    res = bass_utils.run_bass_kernel_spmd(nc, in_maps, core_ids=list(range(N_CORES)))
    out = np.concatenate([res.results[c]["outp"] for c in range(N_CORES)], axis=0)
    return out.astype(np.float32)


# revision 8
# speedup vs baseline: 1.0124x; 1.0124x over previous
"""Trainium2 Bass kernel for nn_ChannelEmbeddingLayers (embedding_lookup).

Strategy: data-parallel over the batch (4096 rows -> 8 cores x 512), embedding
tables replicated per core. Per core, each 128-row batch tile does 200
indirect-DMA gathers (one per feature column, 128 rows each), DVE tree-pools
the sequence features, PE transposes the 2432-wide concat and matmuls against
the replicated dense layer (bias folded in as a constant-1 feature), ACT
applies ReLU straight out of PSUM.

kernel(**inputs) takes the full unsharded inputs and returns the full
[4096, 128] float32 output.
"""
import numpy as np
import concourse.bass as bass
import concourse.mybir as mybir
from concourse import bass_utils
from concourse.masks import make_identity

# Problem shapes (hardcoded per task spec).
B = 4096
N_CORES = 8
BS = B // N_CORES          # 512 rows per core
P = 128                    # batch tile rows
NT = BS // P               # 4 tiles per core
DF, SF = 70, 130           # dense / sparse feature counts
DD, SD = 64, 32            # dense / sparse embedding dims
DSC, SSC = 20, 30          # scalar (non-pooled) feature counts
DSQ, SSQ = DF - DSC, SF - SSC   # 50 / 100 pooled sequence features
V = 1_000_000
# concat layout: [dense scalars 1280][dense pooled 64][sparse scalars 960]
# [sparse pooled 32][ones 1][pad 95] -> K = 2432 = 19*128
OFF_DP = DSC * DD            # 1280
OFF_SS = OFF_DP + DD         # 1344
OFF_SP = OFF_SS + SSC * SD   # 2304
OFF_ONE = OFF_SP + SD        # 2336
K = 2432
KT = K // 128                # 19 contraction tiles
NOUT = 128

_cached = None


def _build():
    nc = bass.Bass()
    dkeys = nc.dram_tensor("dkeys", [BS, DF], mybir.dt.int32, kind="ExternalInput")
    skeys = nc.dram_tensor("skeys", [BS, SF], mybir.dt.int32, kind="ExternalInput")
    dtab = nc.dram_tensor("dtab", [V, DD], mybir.dt.float32, kind="ExternalInput")
    stab = nc.dram_tensor("stab", [V, SD], mybir.dt.float32, kind="ExternalInput")
    wmat = nc.dram_tensor("wmat", [K, NOUT], mybir.dt.float32, kind="ExternalInput")
    outp = nc.dram_tensor("outp", [BS, NOUT], mybir.dt.float32, kind="ExternalOutput")

    f32 = mybir.dt.float32
    i32 = mybir.dt.int32
    from contextlib import ExitStack
    with ExitStack() as ctx:
        e = ctx.enter_context
        kd = [e(nc.sbuf_tensor(f"kd{s}", [P, DF], i32)) for s in range(2)]
        ks = [e(nc.sbuf_tensor(f"ks{s}", [P, SF], i32)) for s in range(2)]
        X = [e(nc.sbuf_tensor(f"X{s}", [P, K], f32)) for s in range(2)]
        XT = [e(nc.sbuf_tensor(f"XT{s}", [P, K], f32)) for s in range(2)]
        dsq = [e(nc.sbuf_tensor(f"dsq{s}", [P, DSQ * DD], f32)) for s in range(2)]
        ssq = [e(nc.sbuf_tensor(f"ssq{s}", [P, SSQ * SD], f32)) for s in range(2)]
        W = e(nc.sbuf_tensor("W", [P, K], f32))
        ident = e(nc.sbuf_tensor("ident", [P, P], f32))
        osb = [e(nc.sbuf_tensor(f"osb{s}", [P, NOUT], f32)) for s in range(2)]
        # full-bank (2KB) PSUM tensors so each lands in its own bank: PE-write
        # and DVE-read of different tensors must never share a bank (P10).
        tp = [e(nc.psum_tensor(f"tp{s}", [P, 512], f32)) for s in range(2)]
        acc = [e(nc.psum_tensor(f"acc{s}", [P, 512], f32)) for s in range(2)]

        # one-shot DMA boundary sems (exact, race-free across the 16 SDMA engines)
        bd = [e(nc.semaphore(f"bd{t}")) for t in range(NT)]
        bs_ = [e(nc.semaphore(f"bs{t}")) for t in range(NT)]
        bx = [e(nc.semaphore(f"bx{t}")) for t in range(NT)]
        kt = [e(nc.semaphore(f"kt{t}")) for t in range(NT)]
        g_dump = e(nc.semaphore("g_dump"))
        w_sem = e(nc.semaphore("w_sem"))
        init_sem = e(nc.semaphore("init_sem"))
        pool_sem = e(nc.semaphore("pool_sem"))
        pe_t_sem = e(nc.semaphore("pe_t_sem"))
        act_c_sem = e(nc.semaphore("act_c_sem"))
        pe_m_sem = e(nc.semaphore("pe_m_sem"))
        act_r_sem = e(nc.semaphore("act_r_sem"))
        store_sem = e(nc.semaphore("store_sem"))

        block = e(nc.Block())

        @block.sync
        def _(sync):
            # keys for the first two tiles go first so the gather stream can
            # start immediately; the W chunks (only needed by PE much later)
            # follow on the same FIFO ring.
            for t in range(2):
                sync.dma_start(kd[t % 2][:], dkeys[t * P:(t + 1) * P, :]).then_inc(g_dump, 16)
                sync.dma_start(ks[t % 2][:], skeys[t * P:(t + 1) * P, :]).then_inc(kt[t], 16)
            # replicated dense layer: 19 chunks of [128, 128]
            for k in range(KT):
                inst = sync.dma_start(W[:, k * 128:(k + 1) * 128],
                                      wmat[k * 128:(k + 1) * 128, :])
                inst.then_inc(w_sem, 16)
            for t in range(2, NT):
                sync.wait_ge(bx[t - 2], 16)  # keys slot free
                sync.dma_start(kd[t % 2][:], dkeys[t * P:(t + 1) * P, :]).then_inc(g_dump, 16)
                sync.dma_start(ks[t % 2][:], skeys[t * P:(t + 1) * P, :]).then_inc(kt[t], 16)
            for t in range(NT):
                sync.wait_ge(act_r_sem, t + 1)
                sync.dma_start(outp[t * P:(t + 1) * P, :], osb[t % 2][:]).then_inc(store_sem, 16)
            sync.wait_ge(store_sem, 16 * NT)

        @block.gpsimd
        def _(gpsimd):
            for s in range(2):
                gpsimd.memset(X[s][:, OFF_ONE:OFF_ONE + 1], 1.0)
                gpsimd.memset(X[s][:, OFF_ONE + 1:K], 0.0)
            make_identity(nc, ident[:])
            gpsimd.sem_inc(init_sem, 1)
            for t in range(NT):
                sl = t % 2
                gpsimd.wait_ge(kt[t], 16)
                if t >= 2:
                    gpsimd.wait_ge(pool_sem, 2 * (t - 1))   # dsq/ssq slot free
                    gpsimd.wait_ge(pe_t_sem, KT * (t - 1))  # X slot free

                def gat(dst, table, keys_sb, col, sem, val):
                    gpsimd.indirect_dma_start(
                        out=dst, out_offset=None, in_=table[:],
                        in_offset=bass.IndirectOffsetOnAxis(
                            ap=keys_sb[:, col:col + 1], axis=0),
                    ).then_inc(sem, val)

                for j in range(DSQ):   # dense sequence -> dsq
                    last = j == DSQ - 1
                    gat(dsq[sl][:, j * DD:(j + 1) * DD], dtab, kd[sl], DSC + j,
                        bd[t] if last else g_dump, 16)
                for j in range(SSQ):   # sparse sequence -> ssq
                    last = j == SSQ - 1
                    gat(ssq[sl][:, j * SD:(j + 1) * SD], stab, ks[sl], SSC + j,
                        bs_[t] if last else g_dump, 16)
                for j in range(DSC):   # dense scalars -> X
                    gat(X[sl][:, j * DD:(j + 1) * DD], dtab, kd[sl], j, g_dump, 16)
                for j in range(SSC):   # sparse scalars -> X
                    last = j == SSC - 1
                    gat(X[sl][:, OFF_SS + j * SD:OFF_SS + (j + 1) * SD], stab, ks[sl], j,
                        bx[t] if last else g_dump, 16)

        @block.vector
        def _(vector):
            for t in range(NT):
                sl = t % 2
                d, s = dsq[sl], ssq[sl]
                if t >= 2:
                    vector.wait_ge(pe_t_sem, KT * (t - 1))  # X slot free for pooled writes
                vector.wait_ge(bd[t], 16)
                # 50 chunks of 64 -> 1
                nc.vector.tensor_add(d[:, 0:1600], d[:, 0:1600], d[:, 1600:3200])
                nc.vector.tensor_add(d[:, 0:768], d[:, 0:768], d[:, 768:1536])
                nc.vector.tensor_add(d[:, 0:384], d[:, 0:384], d[:, 384:768])
                nc.vector.tensor_add(d[:, 0:192], d[:, 0:192], d[:, 192:384])
                nc.vector.tensor_add(d[:, 0:64], d[:, 0:64], d[:, 64:128])
                nc.vector.tensor_add(d[:, 0:64], d[:, 0:64], d[:, 128:192])
                nc.vector.tensor_add(d[:, 0:64], d[:, 0:64], d[:, 1536:1600])
                nc.vector.tensor_scalar_mul(
                    X[sl][:, OFF_DP:OFF_DP + DD], d[:, 0:64], 1.0 / DSQ
                ).then_inc(pool_sem, 1)
                vector.wait_ge(bs_[t], 16)
                # 100 chunks of 32 -> 1
                nc.vector.tensor_add(s[:, 0:1600], s[:, 0:1600], s[:, 1600:3200])
                nc.vector.tensor_add(s[:, 0:800], s[:, 0:800], s[:, 800:1600])
                nc.vector.tensor_add(s[:, 0:384], s[:, 0:384], s[:, 384:768])
                nc.vector.tensor_add(s[:, 0:192], s[:, 0:192], s[:, 192:384])
                nc.vector.tensor_add(s[:, 0:96], s[:, 0:96], s[:, 96:192])
                nc.vector.tensor_add(s[:, 0:32], s[:, 0:32], s[:, 32:64])
                nc.vector.tensor_add(s[:, 0:32], s[:, 0:32], s[:, 64:96])
                nc.vector.tensor_add(s[:, 0:32], s[:, 0:32], s[:, 768:800])
                nc.vector.tensor_scalar_mul(
                    X[sl][:, OFF_SP:OFF_SP + SD], s[:, 0:32], 1.0 / SSQ
                ).then_inc(pool_sem, 1)
                # PSUM->SBUF transpose copies (fast on DVE; ACT would cost
                # ~1.8us each and serialize the kernel tail)
                if t >= 2:
                    vector.wait_ge(pe_m_sem, t - 1)      # XT slot free
                for k in range(KT):
                    vector.wait_ge(pe_t_sem, KT * t + k + 1)
                    nc.vector.tensor_copy(
                        out=XT[sl][:, k * 128:(k + 1) * 128], in_=tp[k % 2][:, 0:P]
                    ).then_inc(act_c_sem, 1)

        @block.tensor
        def _(tensor):
            tensor.wait_ge(init_sem, 1)
            tensor.wait_ge(w_sem, 16 * KT)
            for t in range(NT):
                sl = t % 2
                tensor.wait_ge(bx[t], 16)
                tensor.wait_ge(pool_sem, 2 * t + 2)
                for k in range(KT):
                    thresh = KT * t + k - 1
                    if thresh > 0:
                        tensor.wait_ge(act_c_sem, thresh)  # tp bank k%2 free
                    nc.tensor.transpose(
                        out=tp[k % 2][:, 0:P], in_=X[sl][:, k * 128:(k + 1) * 128],
                        identity=ident[:],
                    ).then_inc(pe_t_sem, 1)
                tensor.wait_ge(act_c_sem, KT * (t + 1))  # all XT chunks ready
                if t >= 2:
                    tensor.wait_ge(act_r_sem, t - 1)     # acc slot free
                for k in range(KT):
                    inst = nc.tensor.matmul(
                        out=acc[t % 2][:, 0:NOUT],
                        lhsT=XT[sl][:, k * 128:(k + 1) * 128],
                        rhs=W[:, k * 128:(k + 1) * 128],
                        start=(k == 0), stop=(k == KT - 1),
                    )
                    if k == KT - 1:
                        inst.then_inc(pe_m_sem, 1)

        @block.scalar
        def _(scalar):
            for t in range(NT):
                scalar.wait_ge(pe_m_sem, t + 1)
                if t >= 2:
                    scalar.wait_ge(store_sem, 16 * (t - 1))  # osb slot free
                nc.scalar.activation(
                    out=osb[t % 2][:], in_=acc[t % 2][:, 0:NOUT],
                    func=mybir.ActivationFunctionType.Relu,
                ).then_inc(act_r_sem, 1)

    nc.finalize()
    return nc


def kernel(dense_keys, sparse_keys, dense_table, sparse_table, dnn_w, dnn_b):
    global _cached
    if _cached is None:
        _cached = _build()
    nc = _cached

    dense_keys = np.asarray(dense_keys)
    sparse_keys = np.asarray(sparse_keys)
    dense_table = np.ascontiguousarray(np.asarray(dense_table, dtype=np.float32))
    sparse_table = np.ascontiguousarray(np.asarray(sparse_table, dtype=np.float32))
    dnn_w = np.asarray(dnn_w, dtype=np.float32)
    dnn_b = np.asarray(dnn_b, dtype=np.float32)

    # fold bias in as a constant-1 feature; zero-pad K to 19*128
    wfull = np.zeros((K, NOUT), np.float32)
    wfull[:OFF_ONE] = dnn_w
    wfull[OFF_ONE] = dnn_b

    dk = np.ascontiguousarray(dense_keys.astype(np.int32))
    sk = np.ascontiguousarray(sparse_keys.astype(np.int32))

    in_maps = []
    for c in range(N_CORES):
        r = slice(c * BS, (c + 1) * BS)
        in_maps.append({
            "dkeys": dk[r], "skeys": sk[r],
            "dtab": dense_table, "stab": sparse_table,
            "wmat": wfull,
        })

    res = bass_utils.run_bass_kernel_spmd(nc, in_maps, core_ids=list(range(N_CORES)))
    out = np.concatenate([res.results[c]["outp"] for c in range(N_CORES)], axis=0)
    return out.astype(np.float32)


# revision 13
# speedup vs baseline: 1.0243x; 1.0118x over previous
"""Trainium2 Bass kernel for nn_ChannelEmbeddingLayers (embedding_lookup).

Strategy: data-parallel over the batch (4096 rows -> 8 cores x 512), embedding
tables replicated per core. Per core, each 128-row batch tile does 200
indirect-DMA gathers (one per feature column, 128 rows each), DVE tree-pools
the sequence features, PE transposes the 2432-wide concat and matmuls against
the replicated dense layer (bias folded in as a constant-1 feature), ACT
applies ReLU straight out of PSUM.

kernel(**inputs) takes the full unsharded inputs and returns the full
[4096, 128] float32 output.
"""
import numpy as np
import concourse.bass as bass
import concourse.mybir as mybir
from concourse import bass_utils
from concourse.masks import make_identity

# Problem shapes (hardcoded per task spec).
B = 4096
N_CORES = 8
BS = B // N_CORES          # 512 rows per core
P = 128                    # batch tile rows
NT = BS // P               # 4 tiles per core
DF, SF = 70, 130           # dense / sparse feature counts
DD, SD = 64, 32            # dense / sparse embedding dims
DSC, SSC = 20, 30          # scalar (non-pooled) feature counts
DSQ, SSQ = DF - DSC, SF - SSC   # 50 / 100 pooled sequence features
V = 1_000_000
# concat layout: [dense scalars 1280][dense pooled 64][sparse scalars 960]
# [sparse pooled 32][ones 1][pad 95] -> K = 2432 = 19*128
OFF_DP = DSC * DD            # 1280
OFF_SS = OFF_DP + DD         # 1344
OFF_SP = OFF_SS + SSC * SD   # 2304
OFF_ONE = OFF_SP + SD        # 2336
K = 2432
KT = K // 128                # 19 contraction tiles
NOUT = 128

_cached = None


def _build():
    nc = bass.Bass()
    dkeys = nc.dram_tensor("dkeys", [BS, DF], mybir.dt.int32, kind="ExternalInput")
    skeys = nc.dram_tensor("skeys", [BS, SF], mybir.dt.int32, kind="ExternalInput")
    dtab = nc.dram_tensor("dtab", [V, DD], mybir.dt.float32, kind="ExternalInput")
    stab = nc.dram_tensor("stab", [V, SD], mybir.dt.float32, kind="ExternalInput")
    wmat = nc.dram_tensor("wmat", [K, NOUT], mybir.dt.float32, kind="ExternalInput")
    outp = nc.dram_tensor("outp", [BS, NOUT], mybir.dt.float32, kind="ExternalOutput")

    f32 = mybir.dt.float32
    i32 = mybir.dt.int32
    from contextlib import ExitStack
    with ExitStack() as ctx:
        e = ctx.enter_context
        kd = [e(nc.sbuf_tensor(f"kd{s}", [P, DF], i32)) for s in range(2)]
        ks = [e(nc.sbuf_tensor(f"ks{s}", [P, SF], i32)) for s in range(2)]
        X = [e(nc.sbuf_tensor(f"X{s}", [P, K], f32)) for s in range(2)]
        XT = [e(nc.sbuf_tensor(f"XT{s}", [P, K], f32)) for s in range(2)]
        dsq = [e(nc.sbuf_tensor(f"dsq{s}", [P, DSQ * DD], f32)) for s in range(2)]
        ssq = [e(nc.sbuf_tensor(f"ssq{s}", [P, SSQ * SD], f32)) for s in range(2)]
        W = e(nc.sbuf_tensor("W", [P, K], f32))
        ident = e(nc.sbuf_tensor("ident", [P, P], f32))
        osb = [e(nc.sbuf_tensor(f"osb{s}", [P, NOUT], f32)) for s in range(2)]
        # full-bank (2KB) PSUM tensors so each lands in its own bank: PE-write
        # and DVE-read of different tensors must never share a bank (P10).
        tp = [e(nc.psum_tensor(f"tp{s}", [P, 512], f32)) for s in range(2)]
        acc = [e(nc.psum_tensor(f"acc{s}", [P, 512], f32)) for s in range(2)]

        # one-shot DMA boundary sems (exact, race-free across the 16 SDMA engines)
        bd = [e(nc.semaphore(f"bd{t}")) for t in range(NT)]
        bs_ = [e(nc.semaphore(f"bs{t}")) for t in range(NT)]
        bx = [e(nc.semaphore(f"bx{t}")) for t in range(NT)]
        kt = [e(nc.semaphore(f"kt{t}")) for t in range(NT)]
        g_dump = e(nc.semaphore("g_dump"))
        w_sem = e(nc.semaphore("w_sem"))
        init_sem = e(nc.semaphore("init_sem"))
        pool_sem = e(nc.semaphore("pool_sem"))
        pe_t_sem = e(nc.semaphore("pe_t_sem"))
        act_c_sem = e(nc.semaphore("act_c_sem"))
        pe_m_sem = e(nc.semaphore("pe_m_sem"))
        act_r_sem = e(nc.semaphore("act_r_sem"))
        store_sem = e(nc.semaphore("store_sem"))

        block = e(nc.Block())

        @block.sync
        def _(sync):
            # keys for the first two tiles go first so the gather stream can
            # start immediately; the W chunks (only needed by PE much later)
            # follow on the same FIFO ring.
            for t in range(2):
                sync.dma_start(kd[t % 2][:], dkeys[t * P:(t + 1) * P, :]).then_inc(g_dump, 16)
                sync.dma_start(ks[t % 2][:], skeys[t * P:(t + 1) * P, :]).then_inc(kt[t], 16)
            # replicated dense layer: 19 chunks of [128, 128]
            for k in range(KT):
                inst = sync.dma_start(W[:, k * 128:(k + 1) * 128],
                                      wmat[k * 128:(k + 1) * 128, :])
                inst.then_inc(w_sem, 16)
            for t in range(2, NT):
                sync.wait_ge(bs_[t - 2], 16)  # keys slot free (sseq is last user)
                sync.dma_start(kd[t % 2][:], dkeys[t * P:(t + 1) * P, :]).then_inc(g_dump, 16)
                sync.dma_start(ks[t % 2][:], skeys[t * P:(t + 1) * P, :]).then_inc(kt[t], 16)
            for t in range(NT):
                sync.wait_ge(act_r_sem, t + 1)
                sync.dma_start(outp[t * P:(t + 1) * P, :], osb[t % 2][:]).then_inc(store_sem, 16)
            sync.wait_ge(store_sem, 16 * NT)

        @block.gpsimd
        def _(gpsimd):
            for s in range(2):
                gpsimd.memset(X[s][:, OFF_ONE:OFF_ONE + 1], 1.0)
                gpsimd.memset(X[s][:, OFF_ONE + 1:K], 0.0)
            make_identity(nc, ident[:])
            gpsimd.sem_inc(init_sem, 1)
            for t in range(NT):
                sl = t % 2
                gpsimd.wait_ge(kt[t], 16)
                if t >= 2:
                    gpsimd.wait_ge(pool_sem, 2 * (t - 1))   # dsq/ssq slot free
                    gpsimd.wait_ge(pe_t_sem, KT * (t - 1))  # X slot free

                def gat(dst, table, keys_sb, col, sem, val):
                    gpsimd.indirect_dma_start(
                        out=dst, out_offset=None, in_=table[:],
                        in_offset=bass.IndirectOffsetOnAxis(
                            ap=keys_sb[:, col:col + 1], axis=0),
                    ).then_inc(sem, val)

                # scalars first, sequences last: 18 of 19 K-chunks become
                # transposable mid-stream, leaving only sparse-pool in the tail
                for j in range(DSC):   # dense scalars -> X
                    gat(X[sl][:, j * DD:(j + 1) * DD], dtab, kd[sl], j, g_dump, 16)
                for j in range(SSC):   # sparse scalars -> X
                    last = j == SSC - 1
                    gat(X[sl][:, OFF_SS + j * SD:OFF_SS + (j + 1) * SD], stab, ks[sl], j,
                        bx[t] if last else g_dump, 16)
                for j in range(DSQ):   # dense sequence -> dsq
                    last = j == DSQ - 1
                    gat(dsq[sl][:, j * DD:(j + 1) * DD], dtab, kd[sl], DSC + j,
                        bd[t] if last else g_dump, 16)
                for j in range(SSQ):   # sparse sequence -> ssq
                    last = j == SSQ - 1
                    gat(ssq[sl][:, j * SD:(j + 1) * SD], stab, ks[sl], SSC + j,
                        bs_[t] if last else g_dump, 16)

        @block.vector
        def _(vector):
            for t in range(NT):
                sl = t % 2
                d, s = dsq[sl], ssq[sl]
                if t >= 2:
                    vector.wait_ge(pe_t_sem, KT * (t - 1))  # X slot free for pooled writes
                vector.wait_ge(bd[t], 16)
                # 50 chunks of 64 -> 1
                nc.vector.tensor_add(d[:, 0:1600], d[:, 0:1600], d[:, 1600:3200])
                nc.vector.tensor_add(d[:, 0:768], d[:, 0:768], d[:, 768:1536])
                nc.vector.tensor_add(d[:, 0:384], d[:, 0:384], d[:, 384:768])
                nc.vector.tensor_add(d[:, 0:192], d[:, 0:192], d[:, 192:384])
                nc.vector.tensor_add(d[:, 0:64], d[:, 0:64], d[:, 64:128])
                nc.vector.tensor_add(d[:, 0:64], d[:, 0:64], d[:, 128:192])
                nc.vector.tensor_add(d[:, 0:64], d[:, 0:64], d[:, 1536:1600])
                nc.vector.tensor_scalar_mul(
                    X[sl][:, OFF_DP:OFF_DP + DD], d[:, 0:64], 1.0 / DSQ
                ).then_inc(pool_sem, 1)
                # PSUM->SBUF copies for chunks 0..17 run mid-stream
                if t >= 2:
                    vector.wait_ge(pe_m_sem, t - 1)      # XT slot free
                for k in range(KT - 1):
                    vector.wait_ge(pe_t_sem, KT * t + k + 1)
                    nc.vector.tensor_copy(
                        out=XT[sl][:, k * 128:(k + 1) * 128], in_=tp[k % 2][:, 0:P]
                    ).then_inc(act_c_sem, 1)
                vector.wait_ge(bs_[t], 16)
                # 100 chunks of 32 -> 1
                nc.vector.tensor_add(s[:, 0:1600], s[:, 0:1600], s[:, 1600:3200])
                nc.vector.tensor_add(s[:, 0:800], s[:, 0:800], s[:, 800:1600])
                nc.vector.tensor_add(s[:, 0:384], s[:, 0:384], s[:, 384:768])
                nc.vector.tensor_add(s[:, 0:192], s[:, 0:192], s[:, 192:384])
                nc.vector.tensor_add(s[:, 0:96], s[:, 0:96], s[:, 96:192])
                nc.vector.tensor_add(s[:, 0:32], s[:, 0:32], s[:, 32:64])
                nc.vector.tensor_add(s[:, 0:32], s[:, 0:32], s[:, 64:96])
                nc.vector.tensor_add(s[:, 0:32], s[:, 0:32], s[:, 768:800])
                nc.vector.tensor_scalar_mul(
                    X[sl][:, OFF_SP:OFF_SP + SD], s[:, 0:32], 1.0 / SSQ
                ).then_inc(pool_sem, 1)
                # final chunk (sparse-pool) copy — the only copy in the tail
                k = KT - 1
                vector.wait_ge(pe_t_sem, KT * t + k + 1)
                nc.vector.tensor_copy(
                    out=XT[sl][:, k * 128:(k + 1) * 128], in_=tp[k % 2][:, 0:P]
                ).then_inc(act_c_sem, 1)

        @block.tensor
        def _(tensor):
            tensor.wait_ge(init_sem, 1)
            tensor.wait_ge(w_sem, 16 * KT)
            for t in range(NT):
                sl = t % 2
                tensor.wait_ge(bx[t], 16)            # X scalar regions ready
                tensor.wait_ge(pool_sem, 2 * t + 1)  # dense pool ready
                for k in range(KT - 1):              # chunks 0..17 mid-stream
                    thresh = KT * t + k - 1
                    if thresh > 0:
                        tensor.wait_ge(act_c_sem, thresh)  # tp bank k%2 free
                    nc.tensor.transpose(
                        out=tp[k % 2][:, 0:P], in_=X[sl][:, k * 128:(k + 1) * 128],
                        identity=ident[:],
                    ).then_inc(pe_t_sem, 1)
                tensor.wait_ge(act_c_sem, KT * t + KT - 1)  # XT chunks 0..17 ready
                if t >= 2:
                    tensor.wait_ge(act_r_sem, t - 1)     # acc slot free
                for k in range(KT - 1):              # matmuls 0..17 mid-stream
                    nc.tensor.matmul(
                        out=acc[t % 2][:, 0:NOUT],
                        lhsT=XT[sl][:, k * 128:(k + 1) * 128],
                        rhs=W[:, k * 128:(k + 1) * 128],
                        start=(k == 0), stop=False,
                    )
                # tail: sparse-pool chunk only (transpose inside the open
                # accumulation group targets tp, not acc — safe on HW)
                k = KT - 1
                tensor.wait_ge(pool_sem, 2 * t + 2)  # sparse pool ready
                nc.tensor.matmul(
                    out=tp[k % 2][:, 0:P], lhsT=X[sl][:, k * 128:(k + 1) * 128],
                    rhs=ident[:], is_transpose=True, skip_group_check=True,
                ).then_inc(pe_t_sem, 1)
                tensor.wait_ge(act_c_sem, KT * (t + 1))  # final XT chunk ready
                nc.tensor.matmul(
                    out=acc[t % 2][:, 0:NOUT],
                    lhsT=XT[sl][:, k * 128:(k + 1) * 128],
                    rhs=W[:, k * 128:(k + 1) * 128],
                    start=False, stop=True, skip_group_check=True,
                ).then_inc(pe_m_sem, 1)

        @block.scalar
        def _(scalar):
            for t in range(NT):
                scalar.wait_ge(pe_m_sem, t + 1)
                if t >= 2:
                    scalar.wait_ge(store_sem, 16 * (t - 1))  # osb slot free
                nc.scalar.activation(
                    out=osb[t % 2][:], in_=acc[t % 2][:, 0:NOUT],
                    func=mybir.ActivationFunctionType.Relu,
                ).then_inc(act_r_sem, 1)

    nc.finalize()
    return nc


def kernel(dense_keys, sparse_keys, dense_table, sparse_table, dnn_w, dnn_b):
    global _cached
    if _cached is None:
        _cached = _build()
    nc = _cached

    dense_keys = np.asarray(dense_keys)
    sparse_keys = np.asarray(sparse_keys)
    dense_table = np.ascontiguousarray(np.asarray(dense_table, dtype=np.float32))
    sparse_table = np.ascontiguousarray(np.asarray(sparse_table, dtype=np.float32))
    dnn_w = np.asarray(dnn_w, dtype=np.float32)
    dnn_b = np.asarray(dnn_b, dtype=np.float32)

    # fold bias in as a constant-1 feature; zero-pad K to 19*128
    wfull = np.zeros((K, NOUT), np.float32)
    wfull[:OFF_ONE] = dnn_w
    wfull[OFF_ONE] = dnn_b

    dk = np.ascontiguousarray(dense_keys.astype(np.int32))
    sk = np.ascontiguousarray(sparse_keys.astype(np.int32))

    in_maps = []
    for c in range(N_CORES):
        r = slice(c * BS, (c + 1) * BS)
        in_maps.append({
            "dkeys": dk[r], "skeys": sk[r],
            "dtab": dense_table, "stab": sparse_table,
            "wmat": wfull,
        })

    res = bass_utils.run_bass_kernel_spmd(nc, in_maps, core_ids=list(range(N_CORES)))
    out = np.concatenate([res.results[c]["outp"] for c in range(N_CORES)], axis=0)
    return out.astype(np.float32)
